# revision 9
# baseline (speedup 1.0000x reference)
"""Trainium2 Bass kernel for batched no-softmax attention.

Reference computation (per batch element b):
    Q = x @ Wq.T + bq            (L, H)
    K = x @ Wk.T + bk            (L, H)
    V = x @ Wv.T + bv            (L, O)
    scores = (Q @ K.T) / sqrt(H) (L, L)
    out = scores @ V             (L, O)

Shapes: B=8, L=2048, D=H=O=768, fp32.

Strategy:
  - Data-parallel over batch: core i handles batch element i (B == n_cores == 8).
  - Host pre-transposes x -> xT (D, L) and weights -> W.T (D, H) so every
    device-side matmul contracts over the partition dimension with no on-chip
    transposes. The 1/sqrt(d) scale is folded into Wq/bq on the host.
  - Matmul operands are stored in bf16 (fp32 PSUM accumulation); fp32 output.
  - Per-core dataflow:
      phase 1: QT[h,l], KT[h,l] (h-major for the scores matmul) and V[l,o]
               (l-major for the out matmul), biases fused into PSUM evacuation.
      phase 2: for each q-chunk of 256 columns:
                 for each k-tile of 128 rows:
                   scoresT[k, q] += KT_ktile.T @ QT_qchunk   (6 h-tiles)
                   out[q, o]     += scoresT_block.T @ V_ktile (accum over k)
"""

import numpy as np
import ml_dtypes

import concourse.bacc as bacc
import concourse.tile as tile
import concourse.mybir as mybir
from concourse.bass_utils import run_bass_kernel_spmd

B, L, D = 8, 2048, 768
NCORES = 8
DT = D // 128   # 6 d-tiles (contraction tiles for projections)
HT = D // 128   # 6 h-tiles
LT = L // 128   # 16 l-tiles
LCH = 512       # l-chunk for projections
NLC = L // LCH  # 4
QCH = 512       # q-chunk for attention
NQC = L // QCH  # 4
OC = 384        # o-chunk (2 chunks of 384 = 768, each <= 512 fp32 psum bank)
NOC = D // OC   # 2

_dt = mybir.dt
_BF16 = _dt.bfloat16
_F32 = _dt.float32

_cached = None


def _build():
    """Build and compile the per-core Bass program (identical on all cores)."""
    nc = bacc.Bacc("TRN2", target_bir_lowering=False, debug=False,
                   num_devices=NCORES)

    xT = nc.dram_tensor("xT", [D, L], _BF16, kind="ExternalInput").ap()
    wq = nc.dram_tensor("wq", [D, D], _BF16, kind="ExternalInput").ap()
    wk = nc.dram_tensor("wk", [D, D], _BF16, kind="ExternalInput").ap()
    wv = nc.dram_tensor("wv", [D, D], _BF16, kind="ExternalInput").ap()
    # biases packed host-side: [:, 0:HT]=bq*s (h-tiled), [:, HT:2HT]=bk,
    # [:, 2HT:2HT+D]=bv broadcast to all 128 partitions
    bias = nc.dram_tensor("bias", [128, 2 * HT + D], _F32,
                          kind="ExternalInput").ap()
    out = nc.dram_tensor("out", [L, D], _F32, kind="ExternalOutput").ap()

    ident = mybir.ActivationFunctionType.Identity

    with tile.TileContext(nc) as tc:
        with (
            tc.tile_pool(name="inp", bufs=1) as inp,
            tc.tile_pool(name="qkv", bufs=1) as qkv,
            tc.tile_pool(name="work", bufs=1) as work,
        ):
            # ---- load inputs (few multi-dim-AP DMAs; first-needed first) ----
            bias_sb = inp.tile([128, 2 * HT + D], _F32, tag="bias",
                               name="bias_sb")
            bq_sb = bias_sb[:, 0:HT]
            bk_sb = bias_sb[:, HT:2 * HT]
            bv_sb = bias_sb[:, 2 * HT:2 * HT + D]
            nc.sync.dma_start(bias_sb[:], bias[:])

            # xt_all[:, d*L + c] = xT[d*128 + p, c]; one DMA per l-chunk
            # covering all 6 d-tiles via a 3-dim access pattern.
            xt_all = inp.tile([128, DT * L], _BF16, tag="xt", name="xt_all")
            xts = [xt_all[:, d * L:(d + 1) * L] for d in range(DT)]
            xT3 = xT.rearrange("(d p) l -> p d l", p=128)
            xt3 = xt_all.rearrange("p (d l) -> p d l", d=DT)

            def load_xt_chunk(lc):
                ls = slice(lc * LCH, (lc + 1) * LCH)
                nc.sync.dma_start(xt3[:, :, ls], xT3[:, :, ls])

            w_sb, w_ap = {}, {"wq": wq, "wk": wk, "wv": wv}

            def load_w(nm):
                t = inp.tile([128, DT * D], _BF16, tag=nm, name=f"{nm}_sb")
                w_sb[nm] = [t[:, d * D:(d + 1) * D] for d in range(DT)]
                nc.sync.dma_start(
                    t.rearrange("p (d c) -> p d c", d=DT),
                    w_ap[nm].rearrange("(d p) c -> p d c", p=128))

            # first phase-1 group (KT, l-chunk 0) needs wk + xt chunk 0
            load_w("wk")
            load_xt_chunk(0)
            load_w("wq")
            load_xt_chunk(1)
            load_w("wv")
            load_xt_chunk(2)
            load_xt_chunk(3)
            wqs, wks, wvs = w_sb["wq"], w_sb["wk"], w_sb["wv"]

            # ---- PE warm-up: junk matmuls during the DMA head so HAM is
            # un-throttled when the real stream starts (results discarded) ----
            with tc.tile_pool(name="ps_w", bufs=1, space="PSUM") as ps_w:
                junk = work.tile([128, 512], _BF16, tag="junk", name="junk")
                nc.gpsimd.memset(junk[:], 0.0)
                for _ in range(12):
                    pw = ps_w.tile([128, 512], _F32, tag="pw", name="pw")
                    nc.tensor.matmul(pw[:], junk[:, 0:128], junk[:],
                                     start=True, stop=True)

            # ---- persistent Q/K/V in SBUF ----
            qts = [qkv.tile([128, L], _BF16, tag=f"qt{h}", name=f"qt{h}")
                   for h in range(HT)]
            kts = [qkv.tile([128, L], _BF16, tag=f"kt{h}", name=f"kt{h}")
                   for h in range(HT)]
            vts = [qkv.tile([128, D], _BF16, tag=f"vt{lt}", name=f"vt{lt}")
                   for lt in range(LT)]

            # ---- phase 1: projections ----
            with tc.tile_pool(name="ps1", bufs=2, space="PSUM") as ps1:
                for lc in range(NLC):
                    l0 = lc * LCH
                    ls = slice(l0, l0 + LCH)
                    # K^T and Q^T chunks: [h=128, LCH] = sum_d WT[d-blk,h-blk].T @ xT[d-blk, lchunk]
                    for wts, outts, bias in ((wks, kts, bk_sb),
                                             (wqs, qts, bq_sb)):
                        for h in range(HT):
                            pp = ps1.tile([128, LCH], _F32, tag="proj",
                                          name="pp")
                            for d in range(DT):
                                nc.tensor.matmul(
                                    pp[:],
                                    wts[d][:, h * 128:(h + 1) * 128],
                                    xts[d][:, ls],
                                    start=(d == 0), stop=(d == DT - 1),
                                )
                            nc.scalar.activation(outts[h][:, ls], pp[:],
                                                 ident, bias=bias[:, h:h + 1])
                    # V tiles: [l=128, OC] = sum_d xT[d-blk, l-blk].T @ WvT[d-blk, ochunk]
                    for lt in range(lc * (LCH // 128), (lc + 1) * (LCH // 128)):
                        for oc in range(NOC):
                            os_ = slice(oc * OC, (oc + 1) * OC)
                            pv = ps1.tile([128, OC], _F32, tag="vproj",
                                          name="pv")
                            for d in range(DT):
                                nc.tensor.matmul(
                                    pv[:],
                                    xts[d][:, lt * 128:(lt + 1) * 128],
                                    wvs[d][:, os_],
                                    start=(d == 0), stop=(d == DT - 1),
                                )
                            nc.vector.tensor_add(vts[lt][:, os_], pv[:],
                                                 bv_sb[:, os_])

            # ---- phase 2: scoresT and out ----
            # q-chunks of 512; per chunk compute scoresT for all 16 k-tiles
            # into bf16 SBUF, then two o-passes (512 + 256 cols) of the out
            # matmul accumulating over k, with PSUM DMA'd straight to DRAM.
            # The o-passes are software-pipelined one q-chunk behind the
            # scores to keep the PE dense across PSUM-bank reuse (WAR).
            with (
                tc.tile_pool(name="ps_s", bufs=2, space="PSUM") as ps_s,
                tc.tile_pool(name="ps_o", bufs=1, space="PSUM") as ps_o,
            ):
                NSUB = QCH // 128           # 4 q-subtiles per chunk
                OCW = (512, 256)            # o-pass widths
                ssbs = [[None] * LT for _ in range(NQC)]

                def emit_scores(qc):
                    q0 = qc * QCH
                    for k in range(LT):
                        sp = ps_s.tile([128, QCH], _F32, tag="sp", name="sp")
                        for h in range(HT):
                            nc.tensor.matmul(
                                sp[:],
                                kts[h][:, k * 128:(k + 1) * 128],
                                qts[h][:, q0:q0 + QCH],
                                start=(h == 0), stop=(h == HT - 1),
                            )
                        ssb = work.tile([128, QCH], _BF16, tag=f"ssb{k}",
                                        name=f"ssb{k}", bufs=2)
                        nc.vector.tensor_copy(ssb[:], sp[:])
                        ssbs[qc][k] = ssb

                def emit_out_pass(qc, oc):
                    q0 = qc * QCH
                    o0 = 512 * oc
                    ow = OCW[oc]
                    for sub in range(NSUB):
                        op = ps_o.tile([128, 512], _F32, tag=f"op{sub}",
                                       name=f"op{sub}")
                        for k in range(LT):
                            nc.tensor.matmul(
                                op[:, :ow],
                                ssbs[qc][k][:, sub * 128:(sub + 1) * 128],
                                vts[k][:, o0:o0 + ow],
                                start=(k == 0), stop=(k == LT - 1),
                            )
                        ob = work.tile([128, 512], _F32, tag=f"ob{sub}",
                                       name=f"ob{sub}", bufs=2)
                        nc.vector.tensor_copy(ob[:, :ow], op[:, :ow])
                        r0 = q0 + sub * 128
                        nc.sync.dma_start(out[r0:r0 + 128, o0:o0 + ow],
                                          ob[:, :ow])

                for qc in range(NQC):
                    emit_scores(qc)
                    if qc > 0:
                        emit_out_pass(qc - 1, 1)
                    emit_out_pass(qc, 0)
                emit_out_pass(NQC - 1, 1)

    nc.compile()
    return nc


def _get_nc():
    global _cached
    if _cached is None:
        _cached = _build()
    return _cached


def _prep_in_maps(x, Wq, bq, Wk, bk, Wv, bv):
    bf16 = ml_dtypes.bfloat16
    s = np.float32(1.0 / np.sqrt(D))
    x = np.asarray(x, dtype=np.float32)
    wq_t = np.ascontiguousarray((np.asarray(Wq, np.float32).T * s)
                                .astype(bf16))
    wk_t = np.ascontiguousarray(np.asarray(Wk, np.float32).T.astype(bf16))
    wv_t = np.ascontiguousarray(np.asarray(Wv, np.float32).T.astype(bf16))
    bias = np.empty((128, 2 * HT + D), np.float32)
    bias[:, 0:HT] = (np.asarray(bq, np.float32) * s).reshape(HT, 128).T
    bias[:, HT:2 * HT] = np.asarray(bk, np.float32).reshape(HT, 128).T
    bias[:, 2 * HT:] = np.broadcast_to(np.asarray(bv, np.float32), (128, D))
    in_maps = []
    for i in range(NCORES):
        xt = np.ascontiguousarray(x[i].T.astype(bf16))
        in_maps.append({
            "xT": xt, "wq": wq_t, "wk": wk_t, "wv": wv_t, "bias": bias,
        })
    return in_maps


def run(x, Wq, bq, Wk, bk, Wv, bv, trace=False):
    """Run the kernel; returns (output, exec_time_ns or None)."""
    nc = _get_nc()
    in_maps = _prep_in_maps(x, Wq, bq, Wk, bk, Wv, bv)
    res = run_bass_kernel_spmd(nc, in_maps, core_ids=list(range(NCORES)),
                               trace=trace)
    outs = np.stack([res.results[i]["out"] for i in range(NCORES)], axis=0)
    return outs.astype(np.float32), res.exec_time_ns


def kernel(x, Wq, bq, Wk, bk, Wv, bv):
    out, _ = run(x, Wq, bq, Wk, bk, Wv, bv, trace=False)
    return out
